# revision 14
# baseline (speedup 1.0000x reference)
"""Trainium2 Bass kernel for nn_LogicLayer (soft logic-gate mixture layer).

Reference computation:
    p = softmax(weights, axis=-1)            # [OUT, 16]
    c = p @ GATE_COEF                        # [OUT, 4]
    a = x[:, idx0]; b = x[:, idx1]           # [B, OUT]
    out = c0 + c1*a + c2*b + c3*a*b

Strategy (data-parallel over batch, 8 cores, 512 rows each):
  Host: fold softmax+coef into c[OUT,4]; build int16 wrapped index tables.
  Device, per core (all-SBUF transposed table, bf16):
    Phase 1: SWDGE cast-DMA x shard [512, 8192] f32 -> bf16 SBUF tiles, then
             SBUF-source identity-index dma_gather(transpose=True) builds
             four quarter-tables xq[bt][128, 64, 128]: xq[bt][p, c, i] =
             x[bt*128+i, c*128+p].  No DRAM round-trip, no PE/ACT/DVE.
             HW limits found empirically: single_packet=True caps a gather
             at 64 TX descriptors/engine (~992 idxs) -> phase-2 gathers use
             single_packet=False; the SWDGE ring holds 1024 descs/engine,
             capping RX spray descs (num_idxs*elem_bytes/256/16 + 2) -> the
             id-gather uses 128-token quarter tables (514 RX descs).
    Phase 2: per j-group, SBUF->SBUF dma_gather(transpose=True) fetches
             a = x[:, idx0], b = x[:, idx1] in natural layout; PE transposes
             them to PSUM (bf16); ACT computes u = c1*a + c0, v = c3*a + c2
             from PSUM with per-partition scalars; DVE w = v*b, o = w + u;
             outT [8192, 512] written to DRAM in bf16.
  Host: upcast + transpose + concat per-core out^T slices into out [4096, 8192].

HBM traffic per core: 16 MiB x read + 8 MiB outT write (vs 80 MiB for the
DRAM-xT f32 variant).
"""

import numpy as np

B, IN_DIM, OUT_DIM = 4096, 8192, 8192
N_CORES = 8
BSH = B // N_CORES  # 512 batch rows per core
NBT = BSH // 128    # 4 batch tiles per core

GATE_COEF = np.array([
    [0.,  0.,  0.,  0.],
    [0.,  0.,  0.,  1.],
    [0.,  1.,  0., -1.],
    [0.,  1.,  0.,  0.],
    [0.,  0.,  1., -1.],
    [0.,  0.,  1.,  0.],
    [0.,  1.,  1., -2.],
    [0.,  1.,  1., -1.],
    [1., -1., -1.,  1.],
    [1., -1., -1.,  2.],
    [1.,  0., -1.,  0.],
    [1.,  0., -1.,  1.],
    [1., -1.,  0.,  0.],
    [1., -1.,  0.,  1.],
    [1.,  0.,  0., -1.],
    [1.,  0.,  0.,  0.],
], dtype=np.float32)

_NC_CACHE = {}


def build_nc(bsh=BSH, in_dim=IN_DIM, out_dim=OUT_DIM, jgroup=2048, fchunk=2048,
             loop_n=1, timing=False, p1_reps=1, p2_reps=1, overlap=False):
    """Build the per-core Bass program (SPMD: same program on all cores).

    timing=True keeps only tiny tensors as external I/O so the per-call
    transfer cost is constant; p1_reps/p2_reps python-unroll the phases for
    slope timing.
    """
    import concourse.bacc as bacc
    import concourse.mybir as mybir
    import concourse.tile as tile
    from concourse.masks import make_identity

    f32 = mybir.dt.float32
    bf16 = mybir.dt.bfloat16
    i16 = mybir.dt.int16
    AF = mybir.ActivationFunctionType
    OP = mybir.AluOpType

    nbt = bsh // 128         # batch tiles (= quarter tables)
    njb = out_dim // 128     # output-column blocks
    jgroup = min(jgroup, out_dim)
    ngr = out_dim // jgroup  # gather groups
    spg = jgroup // 128      # 128-col blocks per group
    icols = jgroup // 16     # idx-table columns per group

    nc = bacc.Bacc("TRN2", target_bir_lowering=False, debug=False)
    big = "Internal" if timing else None
    x = nc.dram_tensor("x", [bsh, in_dim], f32,
                       kind=big or "ExternalInput")
    ctab = nc.dram_tensor("ctab", [128, njb * 4], f32, kind="ExternalInput")
    idx0w = nc.dram_tensor("idx0w", [128, out_dim // 16], i16, kind="ExternalInput")
    idx1w = nc.dram_tensor("idx1w", [128, out_dim // 16], i16, kind="ExternalInput")
    biota = nc.dram_tensor("biota", [128, 8], i16, kind="ExternalInput")
    outT = nc.dram_tensor("outT", [out_dim, bsh], bf16,
                          kind=big or "ExternalOutput")
    dummy = None
    if timing:
        dummy = nc.dram_tensor("tout", [128, 128], f32, kind="ExternalOutput")

    with tile.TileContext(nc) as tc:
        with (
            tc.tile_pool(name="const", bufs=1) as cpool,
            tc.tile_pool(name="xq", bufs=1) as xqpool,
            tc.tile_pool(name="xin", bufs=2) as xpool,
            tc.tile_pool(name="gather", bufs=2) as gpool,
            tc.tile_pool(name="psum", bufs=4, space="PSUM") as pspool,
            tc.tile_pool(name="tmp", bufs=2) as tpool,
            tc.tile_pool(name="out", bufs=1) as opool,
        ):
            ident = cpool.tile([128, 128], bf16)
            make_identity(nc, ident)
            ctab_sb = cpool.tile([128, njb * 4], f32)
            nc.sync.dma_start(ctab_sb, ctab[:, :])
            idx0_sb = cpool.tile([128, out_dim // 16], i16)
            nc.sync.dma_start(idx0_sb, idx0w[:, :])
            idx1_sb = cpool.tile([128, out_dim // 16], i16)
            nc.sync.dma_start(idx1_sb, idx1w[:, :])
            iota_sb = cpool.tile([128, 8], i16)
            nc.sync.dma_start(iota_sb, biota[:, :])

            def phase1():
                """Build the bf16 transposed quarter-tables in SBUF."""
                xqs = []
                for bt in range(nbt):
                    xin = xpool.tile([128, in_dim], bf16, tag="xin")
                    # SWDGE cast f32 -> bf16 during the DMA.
                    nc.gpsimd.dma_start(
                        xin, x[bt * 128:(bt + 1) * 128, :])
                    xq = xqpool.tile([128, in_dim // 128, 128], bf16,
                                     tag=f"xq{bt}")
                    # Identity-index SBUF-source gather: transpose xin into
                    # xq (xq[p, c, i] = xin[i, c*128+p]).
                    nc.gpsimd.dma_gather(
                        xq[:, :, :], xin[:, :], iota_sb[:, :],
                        128, 128, in_dim,
                        transpose=True,
                        sbuf_tokens_per_rank=128,
                        sbuf_free_dim_per_rank=in_dim * 2,
                    )
                    xqs.append(xq)
                return xqs

            def phase2(xqs):
                for g in range(ngr):
                    a_sb = gpool.tile([128, nbt, 1, jgroup], bf16, tag="ga")
                    b_sb = gpool.tile([128, nbt, 1, jgroup], bf16, tag="gb")
                    for t in range(nbt):
                        nc.gpsimd.dma_gather(
                            a_sb[:, t], xqs[t][:, :, :],
                            idx0_sb[:, g * icols:(g + 1) * icols],
                            jgroup, jgroup, 128,
                            transpose=True,
                            single_packet=False,
                            sbuf_tokens_per_rank=128,
                            sbuf_free_dim_per_rank=256,
                        )
                        nc.gpsimd.dma_gather(
                            b_sb[:, t], xqs[t][:, :, :],
                            idx1_sb[:, g * icols:(g + 1) * icols],
                            jgroup, jgroup, 128,
                            transpose=True,
                            single_packet=False,
                            sbuf_tokens_per_rank=128,
                            sbuf_free_dim_per_rank=256,
                        )
                    o_sb = opool.tile([128, spg, bsh], bf16, tag="go")
                    for s in range(spg):
                        jb = g * spg + s
                        psA = pspool.tile([128, bsh], bf16, tag="psA")
                        psB = pspool.tile([128, bsh], bf16, tag="psB")
                        for t in range(nbt):
                            nc.tensor.transpose(
                                psA[:, t * 128:(t + 1) * 128],
                                a_sb[:, t, 0, s * 128:(s + 1) * 128],
                                ident,
                            )
                            nc.tensor.transpose(
                                psB[:, t * 128:(t + 1) * 128],
                                b_sb[:, t, 0, s * 128:(s + 1) * 128],
                                ident,
                            )
                        u = tpool.tile([128, bsh], bf16, tag="u")
                        v = tpool.tile([128, bsh], bf16, tag="v")
                        w = tpool.tile([128, bsh], bf16, tag="w")
                        # u = c1*a + c0 ; v = c3*a + c2 (per-partition scalars)
                        nc.scalar.activation(
                            u, psA, AF.Identity,
                            bias=ctab_sb[:, jb * 4 + 0:jb * 4 + 1],
                            scale=ctab_sb[:, jb * 4 + 1:jb * 4 + 2],
                        )
                        nc.scalar.activation(
                            v, psA, AF.Identity,
                            bias=ctab_sb[:, jb * 4 + 2:jb * 4 + 3],
                            scale=ctab_sb[:, jb * 4 + 3:jb * 4 + 4],
                        )
                        nc.vector.tensor_tensor(w, v, psB, OP.mult)
                        nc.vector.tensor_tensor(o_sb[:, s], w, u, OP.add)
                    og = outT[g * jgroup:(g + 1) * jgroup, :].rearrange(
                        "(s p) c -> p s c", p=128
                    )
                    nc.sync.dma_start(og, o_sb[:, :, :])

            def body():
                xqs = None
                for _p1 in range(p1_reps):
                    xqs = phase1()
                for _p2 in range(p2_reps):
                    phase2(xqs)

            if loop_n > 1:
                with tc.For_i(0, loop_n) as _i:
                    body()
            else:
                body()

            if dummy is not None:
                nc.sync.dma_start(dummy[:, :], ctab_sb[:, 0:128])

    nc.compile()
    return nc


def host_prep(weights, idx0, idx1, out_dim=OUT_DIM):
    """Fold softmax+gate coefficients; build wrapped int16 index tables."""
    w = np.asarray(weights, dtype=np.float32)
    m = w.max(axis=-1, keepdims=True)
    e = np.exp(w - m, dtype=np.float32)
    p = e / e.sum(axis=-1, keepdims=True, dtype=np.float32)
    c = (p @ GATE_COEF).astype(np.float32)  # [out_dim, 4]
    njb = out_dim // 128
    # ctab[p, jb*4+k] = c[jb*128+p, k]
    ctab = np.ascontiguousarray(
        c.reshape(njb, 128, 4).transpose(1, 0, 2).reshape(128, njb * 4)
    )

    def wrap(idx, cols16=16):
        idx = np.asarray(idx).astype(np.int16)
        n = idx.size
        t = idx.reshape(n // 16, 16).T  # [16, cols]; t[p, col] = idx[col*16+p]
        return np.ascontiguousarray(np.tile(t, (8, 1)))  # replicate to 128 parts

    biota = wrap(np.arange(128, dtype=np.int16))
    return ctab, wrap(idx0), wrap(idx1), biota


def kernel(x, weights, idx0, idx1):
    from concourse.bass_utils import run_bass_kernel_spmd

    x = np.ascontiguousarray(np.asarray(x, dtype=np.float32))
    ctab, i0w, i1w, biota = host_prep(weights, idx0, idx1)

    if "nc" not in _NC_CACHE:
        _NC_CACHE["nc"] = build_nc()
    nc = _NC_CACHE["nc"]

    in_maps = [
        {
            "x": x[c * BSH:(c + 1) * BSH],
            "ctab": ctab,
            "idx0w": i0w,
            "idx1w": i1w,
            "biota": biota,
        }
        for c in range(N_CORES)
    ]
    res = run_bass_kernel_spmd(nc, in_maps, core_ids=list(range(N_CORES)))
    out = np.empty((B, OUT_DIM), dtype=np.float32)
    for c in range(N_CORES):
        out[c * BSH:(c + 1) * BSH] = res.results[c]["outT"].astype(np.float32).T
    return out
